# revision 20
# baseline (speedup 1.0000x reference)
"""MoE (top-2, 8 experts) SwiGLU MLP — Trainium2 Bass kernel.

Contract: kernel(**inputs) takes the FULL unsharded inputs (numpy or jax
arrays) and returns the full output, matching reference():
    (y: (8, 2048, 1024) fp32, aux: scalar fp32)

Sharding: expert-parallel across the 8 NeuronCores. The gate (tiny) is
computed on host with the exact same jnp op sequence as the reference
(bit-identical routing); tokens are dispatched to their top-2 experts'
cores (the all-to-all is host-side since we hold full inputs), each core
runs a dense SwiGLU MLP for its expert over its routed tokens (bf16
matmuls, fp32 accumulation), and the host combines the per-expert
outputs with the gate weights.

Device data layouts are pre-arranged on host so every DMA is a single
contiguous run per SBUF partition:
  xt: (CO, P, cap)      token chunk co-slices stream as [P, tc]
  w1/w3: (HSL, P, CO, HS)  resident, loaded in H-slices for fast startup
  w2: (HO, P, C)        streamed per h-tile during the down-projection
"""

import os
import sys

import numpy as np

if "/opt/trn_rl_repo" not in sys.path:
    sys.path.insert(0, "/opt/trn_rl_repo")

B, T, C, E, K, H = 8, 2048, 1024, 8, 2, 4096
S = B * T
P = 128
TC = 512  # token chunk per pipeline stage
CO = C // P  # 8
HO = H // P  # 32
HSL = 8  # weight H-slices
HS = H // HSL  # 512

_cache = {}
last_exec_time_ns = None


def _chunks_for(cap):
    """Chunk schedule. A trailing remainder < TC is split across the last
    two chunks (e.g. [512..., 512, 256] -> [512..., 384, 384]): a small tail
    chunk's down-projection re-streams the full W2 over too few tokens and
    exceeds the ~358 GB/s per-core HBM bandwidth; two mid-size chunks stay
    comfortably under it. Chunk sizes must be multiples of 128."""
    sizes = []
    rem = cap
    while rem > 0:
        tc = min(TC, rem)
        sizes.append(tc)
        rem -= tc
    if len(sizes) >= 2 and sizes[-1] < TC:
        pair = sizes[-2] + sizes[-1]
        a = (pair // 2 + P - 1) // P * P
        sizes[-2:] = [a, pair - a]
    out = []
    t0 = 0
    for tc in sizes:
        out.append((t0, tc))
        t0 += tc
    return out


def _build_program(cap, use_b1, use_b3):
    key = (cap, use_b1, use_b3)
    if key in _cache:
        return _cache[key]

    import concourse.bass as bass  # noqa: F401
    import concourse.mybir as mybir
    from concourse import bacc
    from concourse.tile import TileContext

    f32 = mybir.dt.float32
    bf16 = mybir.dt.bfloat16
    AF = mybir.ActivationFunctionType

    nc = bacc.Bacc(None, target_bir_lowering=False)
    xt = nc.declare_dram_parameter("xt", [CO, P, cap], bf16, isOutput=False)
    w1 = nc.declare_dram_parameter("w1", [HSL, P, CO, HS], bf16, isOutput=False)
    w3 = nc.declare_dram_parameter("w3", [HSL, P, CO, HS], bf16, isOutput=False)
    w2 = nc.declare_dram_parameter("w2", [HO, P, C], bf16, isOutput=False)
    if use_b1:
        b1 = nc.declare_dram_parameter("b1", [H], f32, isOutput=False)
    if use_b3:
        b3 = nc.declare_dram_parameter("b3", [H], f32, isOutput=False)
    out = nc.declare_dram_parameter("out", [cap, C], f32, isOutput=True)

    chunks = _chunks_for(cap)
    HT_PER_SL = HS // P  # 4 h-tiles per weight slice

    with TileContext(nc) as tc:
        with (
            tc.tile_pool(name="wconst", bufs=1) as wconst,
            tc.tile_pool(name="xtp", bufs=2) as xtp,
            tc.tile_pool(name="gtp", bufs=1) as gtp,
            tc.tile_pool(name="w2p", bufs=7) as w2p,
            tc.tile_pool(name="actp", bufs=1) as actp,
            tc.tile_pool(name="yp", bufs=3) as yp,
            tc.tile_pool(name="ps1", bufs=2, space="PSUM") as ps1,
            tc.tile_pool(name="ps2", bufs=4, space="PSUM") as ps2,
        ):
            def alloc_xts(t0, tcn, ci, defer_dma=False):
                tiles = [
                    xtp.tile([P, TC], bf16, tag=f"xts{co}", name=f"xts_{ci}_{co}")
                    for co in range(CO)
                ]
                if not defer_dma:
                    for co in range(CO):
                        nc.sync.dma_start(
                            out=tiles[co][:, :tcn], in_=xt[co, :, t0 : t0 + tcn]
                        )
                return tiles

            # Startup: the very first matmul needs only xts[0] and the co=0
            # piece of the first weight slice. Slice the first H-slice of
            # w1/w3 by co and interleave the DMAs in consumption order so the
            # PE starts within a few microseconds instead of waiting for the
            # full 4 MB slice pair.
            xts0 = alloc_xts(chunks[0][0], chunks[0][1], 0, defer_dma=True)
            w1s0 = [wconst.tile([P, HS], bf16, name=f"w1s0co{co}") for co in range(CO)]
            w3s0 = [wconst.tile([P, HS], bf16, name=f"w3s0co{co}") for co in range(CO)]
            tcn0 = chunks[0][1]
            for co in range(CO):
                nc.sync.dma_start(out=xts0[co][:, :tcn0], in_=xt[co, :, 0:tcn0])
                nc.sync.dma_start(out=w1s0[co][:], in_=w1[0, :, co, :])
                nc.sync.dma_start(out=w3s0[co][:], in_=w3[0, :, co, :])

            w1s = [None] + [
                wconst.tile([P, CO, HS], bf16, name=f"w1s{i}") for i in range(1, HSL)
            ]
            w3s = [None] + [
                wconst.tile([P, CO, HS], bf16, name=f"w3s{i}") for i in range(1, HSL)
            ]
            for i in range(1, HSL):
                nc.sync.dma_start(out=w1s[i][:], in_=w1[i])
                nc.sync.dma_start(out=w3s[i][:], in_=w3[i])
            W2RES = 8  # leading ho-tiles of W2 kept resident (saves 2 MB DMA/chunk)
            w2r = wconst.tile([P, W2RES, C], bf16, name="w2r")
            nc.sync.dma_start(out=w2r[:], in_=w2[0:W2RES].rearrange("o p c -> p o c"))
            if use_b1:
                b1sb = wconst.tile([P, HO], f32)
                nc.sync.dma_start(out=b1sb[:], in_=b1.rearrange("(ho hi) -> hi ho", hi=P))
            if use_b3:
                b3sb = wconst.tile([P, HO], f32)
                nc.sync.dma_start(out=b3sb[:], in_=b3.rearrange("(ho hi) -> hi ho", hi=P))

            for ci, (t0, tcn) in enumerate(chunks):
                xts = xts0 if ci == 0 else alloc_xts(t0, tcn, ci)

                # ---- stage 1: GT[h, t] = silu(W1^T x + b1) * (W3^T x + b3) ----
                # The c2=0 half of stage 2 is interleaved here, lagged one ht
                # behind (its gts row is complete and its W2 stream spreads
                # over the whole stage-1 window instead of bursting later).
                n_tt = tcn // P
                psy0 = [
                    ps2.tile([P, TC], f32, tag="psy", name=f"psy0_{tt}")
                    for tt in range(n_tt)
                ]

                def s2_c20(ho):
                    if ho < W2RES:
                        w2t = w2r[:, ho, 0:512]
                    else:
                        w2t = w2p.tile([P, 512], bf16, tag="w2t")
                        nc.sync.dma_start(out=w2t[:], in_=w2[ho, :, 0:512])
                    for tt in range(n_tt):
                        nc.tensor.matmul(
                            psy0[tt][:, :512],
                            lhsT=gts[:, ho, tt * P : (tt + 1) * P],
                            rhs=w2t[:],
                            start=(ho == 0),
                            stop=(ho == HO - 1),
                        )

                gts = gtp.tile([P, HO, TC], bf16, tag="gts")
                for ht in range(HO):
                    wsl = ht // HT_PER_SL
                    hof = (ht % HT_PER_SL) * P
                    p1 = ps1.tile([P, TC], f32, tag="p1")
                    p3 = ps1.tile([P, TC], f32, tag="p3")
                    for co in range(CO):
                        lhs1 = (w1s0[co][:, hof : hof + P] if wsl == 0
                                else w1s[wsl][:, co, hof : hof + P])
                        nc.tensor.matmul(
                            p1[:, :tcn],
                            lhsT=lhs1,
                            rhs=xts[co][:, :tcn],
                            start=(co == 0),
                            stop=(co == CO - 1),
                        )
                    for co in range(CO):
                        lhs3 = (w3s0[co][:, hof : hof + P] if wsl == 0
                                else w3s[wsl][:, co, hof : hof + P])
                        nc.tensor.matmul(
                            p3[:, :tcn],
                            lhsT=lhs3,
                            rhs=xts[co][:, :tcn],
                            start=(co == 0),
                            stop=(co == CO - 1),
                        )
                    if ht >= 1:
                        s2_c20(ht - 1)
                    tmp = actp.tile([P, TC], f32, tag="silu")
                    if use_b1:
                        nc.scalar.activation(
                            tmp[:, :tcn], p1[:, :tcn], AF.Silu, bias=b1sb[:, ht : ht + 1]
                        )
                    else:
                        nc.scalar.activation(tmp[:, :tcn], p1[:, :tcn], AF.Silu)
                    if use_b3:
                        h3t = actp.tile([P, TC], f32, tag="h3t")
                        nc.vector.tensor_scalar_add(
                            h3t[:, :tcn], p3[:, :tcn], b3sb[:, ht : ht + 1]
                        )
                        nc.vector.tensor_mul(gts[:, ht, :tcn], tmp[:, :tcn], h3t[:, :tcn])
                    else:
                        nc.vector.tensor_mul(gts[:, ht, :tcn], tmp[:, :tcn], p3[:, :tcn])

                # ---- stage 2 epilogue: finish c2=0, then the c2=1 pass ----
                s2_c20(HO - 1)
                for tt in range(n_tt):
                    ysb = yp.tile([P, 512], f32, tag="ysb")
                    # ACT is idle right after the last silu; evicting here
                    # releases the psy0 banks faster than the DVE (which is
                    # still finishing the last GT multiply)
                    nc.scalar.activation(ysb[:], psy0[tt][:, :512], AF.Copy)
                    nc.sync.dma_start(
                        out=out[t0 + tt * P : t0 + (tt + 1) * P, 0:512],
                        in_=ysb[:],
                    )

                psy1 = [
                    ps2.tile([P, TC], f32, tag="psy", name=f"psy1_{tt}")
                    for tt in range(n_tt)
                ]
                for ho in range(HO):
                    if ho < W2RES:
                        w2t = w2r[:, ho, 512:1024]
                    else:
                        w2t = w2p.tile([P, 512], bf16, tag="w2t")
                        nc.sync.dma_start(out=w2t[:], in_=w2[ho, :, 512:1024])
                    for tt in range(n_tt):
                        nc.tensor.matmul(
                            psy1[tt][:, :512],
                            lhsT=gts[:, ho, tt * P : (tt + 1) * P],
                            rhs=w2t[:],
                            start=(ho == 0),
                            stop=(ho == HO - 1),
                        )
                for tt in range(n_tt):
                    ysb = yp.tile([P, 512], f32, tag="ysb")
                    nc.vector.tensor_copy(out=ysb[:], in_=psy1[tt][:, :512])
                    nc.sync.dma_start(
                        out=out[t0 + tt * P : t0 + (tt + 1) * P, 512:1024],
                        in_=ysb[:],
                    )

    nc.compile()
    _cache[key] = nc
    return nc


def kernel(**inputs):
    global last_exec_time_ns
    import jax
    import jax.numpy as jnp
    import ml_dtypes

    from concourse.bass_utils import run_bass_kernel_spmd

    x = inputs["x"]
    Wg = inputs["Wg"]
    bg = inputs["bg"]
    W1 = np.asarray(inputs["W1"], dtype=np.float32)
    b1 = np.asarray(inputs["b1"], dtype=np.float32)
    W3 = np.asarray(inputs["W3"], dtype=np.float32)
    b3 = np.asarray(inputs["b3"], dtype=np.float32)
    W2 = np.asarray(inputs["W2"], dtype=np.float32)
    b2 = np.asarray(inputs["b2"], dtype=np.float32)

    # ---- gate: exact replication of the reference's jnp op sequence ----
    xj = jnp.asarray(x)
    b, t, c = xj.shape
    xf = xj.reshape(b * t, c)
    logits = xf @ jnp.asarray(Wg) + jnp.asarray(bg)
    probs = jax.nn.softmax(logits, axis=-1)
    w, idx = jax.lax.top_k(probs, K)
    w = w / jnp.sum(w, axis=-1, keepdims=True)
    onehot = jax.nn.one_hot(idx, E, dtype=xf.dtype)
    f = onehot.reshape(-1, E).mean(axis=0)
    Pm = probs.mean(axis=0)
    aux = E * jnp.sum(f * Pm)

    idx_np = np.asarray(idx)
    w_np = np.asarray(w, dtype=np.float32)
    aux_np = np.asarray(aux)

    # ---- dispatch (host-side all-to-all) ----
    xf_np = np.asarray(xf, dtype=np.float32)
    xf_bf = xf_np.astype(ml_dtypes.bfloat16)

    ids = []
    wts = []
    for e in range(E):
        sel0 = np.nonzero(idx_np[:, 0] == e)[0]
        sel1 = np.nonzero(idx_np[:, 1] == e)[0]
        ids.append(np.concatenate([sel0, sel1]))
        wts.append(np.concatenate([w_np[sel0, 0], w_np[sel1, 1]]))
    cap = max(len(i) for i in ids)
    cap = max(TC, ((cap + P - 1) // P) * P)

    use_b1 = bool(np.any(b1))
    use_b3 = bool(np.any(b3))
    nc = _build_program(cap, use_b1, use_b3)

    bf = ml_dtypes.bfloat16
    # device-friendly weight layouts (see module docstring)
    W1r = np.ascontiguousarray(
        W1.astype(bf).reshape(E, CO, P, HSL, HS).transpose(0, 3, 2, 1, 4)
    )
    W3r = np.ascontiguousarray(
        W3.astype(bf).reshape(E, CO, P, HSL, HS).transpose(0, 3, 2, 1, 4)
    )
    W2r = W2.astype(bf).reshape(E, HO, P, C)

    in_maps = []
    for e in range(E):
        xt = np.zeros((C, cap), dtype=bf)
        n_e = len(ids[e])
        xt[:, :n_e] = xf_bf[ids[e]].T
        m = {
            "xt": xt.reshape(CO, P, cap),
            "w1": W1r[e],
            "w3": W3r[e],
            "w2": W2r[e],
        }
        if use_b1:
            m["b1"] = b1[e]
        if use_b3:
            m["b3"] = b3[e]
        in_maps.append(m)

    trace = os.environ.get("MOE_TRACE") == "1"
    kw = {}
    if trace:
        kw["trace"] = True
        kw["tmpdir"] = os.environ.get("MOE_TRACE_DIR") or None
    res = run_bass_kernel_spmd(nc, in_maps, list(range(E)), **kw)
    if trace:
        last_exec_time_ns = res.exec_time_ns

    # ---- combine (host-side) ----
    yflat = np.zeros((S, C), dtype=np.float32)
    for e in range(E):
        n_e = len(ids[e])
        contrib = res.results[e]["out"][:n_e]
        if np.any(b2[e]):
            contrib = contrib + b2[e][None, :]
        yflat[ids[e]] += wts[e][:, None] * contrib

    return yflat.reshape(B, T, C), aux_np


# revision 25
# speedup vs baseline: 1.0087x; 1.0087x over previous
"""MoE (top-2, 8 experts) SwiGLU MLP — Trainium2 Bass kernel.

Contract: kernel(**inputs) takes the FULL unsharded inputs (numpy or jax
arrays) and returns the full output, matching reference():
    (y: (8, 2048, 1024) fp32, aux: scalar fp32)

Sharding: expert-parallel across the 8 NeuronCores. The gate (tiny) is
computed on host with the exact same jnp op sequence as the reference
(bit-identical routing); tokens are dispatched to their top-2 experts'
cores (the all-to-all is host-side since we hold full inputs), each core
runs a dense SwiGLU MLP for its expert over its routed tokens (bf16
matmuls, fp32 accumulation), and the host combines the per-expert
outputs with the gate weights.

Device data layouts are pre-arranged on host so every DMA is a single
contiguous run per SBUF partition:
  xt: (CO, P, cap)      token chunk co-slices stream as [P, tc]
  w1/w3: (HSL, P, CO, HS)  resident, loaded in H-slices for fast startup
  w2: (HO, P, C)        streamed per h-tile during the down-projection
"""

import os
import sys

import numpy as np

if "/opt/trn_rl_repo" not in sys.path:
    sys.path.insert(0, "/opt/trn_rl_repo")

B, T, C, E, K, H = 8, 2048, 1024, 8, 2, 4096
S = B * T
P = 128
TC = 512  # token chunk per pipeline stage
CO = C // P  # 8
HO = H // P  # 32
HSL = 8  # weight H-slices
HS = H // HSL  # 512

_cache = {}
last_exec_time_ns = None


def _chunks_for(cap):
    """Chunk schedule. A trailing remainder < TC is split across the last
    two chunks (e.g. [512..., 512, 256] -> [512..., 384, 384]): a small tail
    chunk's down-projection re-streams the full W2 over too few tokens and
    exceeds the ~358 GB/s per-core HBM bandwidth; two mid-size chunks stay
    comfortably under it. Chunk sizes must be multiples of 128."""
    sizes = []
    rem = cap
    while rem > 0:
        tc = min(TC, rem)
        sizes.append(tc)
        rem -= tc
    if len(sizes) >= 2 and sizes[-1] < TC:
        pair = sizes[-2] + sizes[-1]
        a = (pair // 2 + P - 1) // P * P
        sizes[-2:] = [a, pair - a]
    out = []
    t0 = 0
    for tc in sizes:
        out.append((t0, tc))
        t0 += tc
    return out


def _build_program(cap, use_b1, use_b3):
    key = (cap, use_b1, use_b3)
    if key in _cache:
        return _cache[key]

    import concourse.bass as bass  # noqa: F401
    import concourse.mybir as mybir
    from concourse import bacc
    from concourse.tile import TileContext

    f32 = mybir.dt.float32
    bf16 = mybir.dt.bfloat16
    AF = mybir.ActivationFunctionType

    nc = bacc.Bacc(None, target_bir_lowering=False)
    xt = nc.declare_dram_parameter("xt", [CO, P, cap], bf16, isOutput=False)
    w1 = nc.declare_dram_parameter("w1", [HSL, P, CO, HS], bf16, isOutput=False)
    w3 = nc.declare_dram_parameter("w3", [HSL, P, CO, HS], bf16, isOutput=False)
    w2 = nc.declare_dram_parameter("w2", [HO, P, C], bf16, isOutput=False)
    if use_b1:
        b1 = nc.declare_dram_parameter("b1", [H], f32, isOutput=False)
    if use_b3:
        b3 = nc.declare_dram_parameter("b3", [H], f32, isOutput=False)
    out = nc.declare_dram_parameter("out", [cap, C], f32, isOutput=True)

    chunks = _chunks_for(cap)
    HT_PER_SL = HS // P  # 4 h-tiles per weight slice

    with TileContext(nc) as tc:
        with (
            tc.tile_pool(name="wconst", bufs=1) as wconst,
            tc.tile_pool(name="xtp", bufs=2) as xtp,
            tc.tile_pool(name="gtp", bufs=1) as gtp,
            tc.tile_pool(name="w2p", bufs=16) as w2p,
            tc.tile_pool(name="actp", bufs=2) as actp,
            tc.tile_pool(name="yp", bufs=4) as yp,
            tc.tile_pool(name="ps1", bufs=2, space="PSUM") as ps1,
            tc.tile_pool(name="ps2", bufs=4, space="PSUM") as ps2,
        ):
            def alloc_xts(t0, tcn, ci, defer_dma=False):
                tiles = [
                    xtp.tile([P, TC], bf16, tag=f"xts{co}", name=f"xts_{ci}_{co}")
                    for co in range(CO)
                ]
                if not defer_dma:
                    for co in range(CO):
                        nc.sync.dma_start(
                            out=tiles[co][:, :tcn], in_=xt[co, :, t0 : t0 + tcn]
                        )
                return tiles

            # Startup: the very first matmul needs only xts[0] and the co=0
            # piece of the first weight slice. Slice the first H-slice of
            # w1/w3 by co and interleave the DMAs in consumption order so the
            # PE starts within a few microseconds instead of waiting for the
            # full 4 MB slice pair.
            xts0 = alloc_xts(chunks[0][0], chunks[0][1], 0, defer_dma=True)
            w1s0 = [wconst.tile([P, HS], bf16, name=f"w1s0co{co}") for co in range(CO)]
            w3s0 = [wconst.tile([P, HS], bf16, name=f"w3s0co{co}") for co in range(CO)]
            tcn0 = chunks[0][1]
            for co in range(CO):
                nc.sync.dma_start(out=xts0[co][:, :tcn0], in_=xt[co, :, 0:tcn0])
                nc.sync.dma_start(out=w1s0[co][:], in_=w1[0, :, co, :])
                nc.sync.dma_start(out=w3s0[co][:], in_=w3[0, :, co, :])

            w1s = [None] + [
                wconst.tile([P, CO, HS], bf16, name=f"w1s{i}") for i in range(1, HSL)
            ]
            w3s = [None] + [
                wconst.tile([P, CO, HS], bf16, name=f"w3s{i}") for i in range(1, HSL)
            ]
            for i in range(1, HSL):
                nc.sync.dma_start(out=w1s[i][:], in_=w1[i])
                nc.sync.dma_start(out=w3s[i][:], in_=w3[i])
            # One resident ho-tile of W2 (fits the 3.84 KB SBUF slack): the
            # boundary-critical first matmul of each c2 pass becomes
            # DMA-independent.
            W2RES = 1
            w2r = wconst.tile([P, W2RES, C], bf16, name="w2r")
            nc.sync.dma_start(out=w2r[:], in_=w2[0:W2RES].rearrange("o p c -> p o c"))
            if use_b1:
                b1sb = wconst.tile([P, HO], f32)
                nc.sync.dma_start(out=b1sb[:], in_=b1.rearrange("(ho hi) -> hi ho", hi=P))
            if use_b3:
                b3sb = wconst.tile([P, HO], f32)
                nc.sync.dma_start(out=b3sb[:], in_=b3.rearrange("(ho hi) -> hi ho", hi=P))

            for ci, (t0, tcn) in enumerate(chunks):
                xts = xts0 if ci == 0 else alloc_xts(t0, tcn, ci)

                # ---- stage 1: GT[h, t] = silu(W1^T x + b1) * (W3^T x + b3) ----
                # The c2=0 half of stage 2 is interleaved here, lagged one ht
                # behind (its gts row is complete and its W2 stream spreads
                # over the whole stage-1 window instead of bursting later).
                n_tt = tcn // P
                psy0 = [
                    ps2.tile([P, TC], f32, tag="psy", name=f"psy0_{tt}")
                    for tt in range(n_tt)
                ]

                def s2_c20(ho):
                    if ho < W2RES:
                        w2t = w2r[:, ho, 0:512]
                    else:
                        w2t = w2p.tile([P, 512], bf16, tag="w2t")
                        nc.sync.dma_start(out=w2t[:], in_=w2[ho, :, 0:512])
                    for tt in range(n_tt):
                        nc.tensor.matmul(
                            psy0[tt][:, :512],
                            lhsT=gts[:, ho, tt * P : (tt + 1) * P],
                            rhs=w2t[:],
                            start=(ho == 0),
                            stop=(ho == HO - 1),
                        )

                gts = gtp.tile([P, HO, TC], bf16, tag="gts")
                for ht in range(HO):
                    wsl = ht // HT_PER_SL
                    hof = (ht % HT_PER_SL) * P
                    p1 = ps1.tile([P, TC], f32, tag="p1")
                    p3 = ps1.tile([P, TC], f32, tag="p3")
                    for co in range(CO):
                        lhs1 = (w1s0[co][:, hof : hof + P] if wsl == 0
                                else w1s[wsl][:, co, hof : hof + P])
                        nc.tensor.matmul(
                            p1[:, :tcn],
                            lhsT=lhs1,
                            rhs=xts[co][:, :tcn],
                            start=(co == 0),
                            stop=(co == CO - 1),
                        )
                    for co in range(CO):
                        lhs3 = (w3s0[co][:, hof : hof + P] if wsl == 0
                                else w3s[wsl][:, co, hof : hof + P])
                        nc.tensor.matmul(
                            p3[:, :tcn],
                            lhsT=lhs3,
                            rhs=xts[co][:, :tcn],
                            start=(co == 0),
                            stop=(co == CO - 1),
                        )
                    if ht >= 1:
                        s2_c20(ht - 1)
                    tmp = actp.tile([P, TC], f32, tag="silu")
                    if use_b1:
                        nc.scalar.activation(
                            tmp[:, :tcn], p1[:, :tcn], AF.Silu, bias=b1sb[:, ht : ht + 1]
                        )
                    else:
                        nc.scalar.activation(tmp[:, :tcn], p1[:, :tcn], AF.Silu)
                    if use_b3:
                        h3t = actp.tile([P, TC], f32, tag="h3t")
                        nc.vector.tensor_scalar_add(
                            h3t[:, :tcn], p3[:, :tcn], b3sb[:, ht : ht + 1]
                        )
                        nc.vector.tensor_mul(gts[:, ht, :tcn], tmp[:, :tcn], h3t[:, :tcn])
                    else:
                        nc.vector.tensor_mul(gts[:, ht, :tcn], tmp[:, :tcn], p3[:, :tcn])

                # ---- stage 2 epilogue: finish c2=0, then the c2=1 pass ----
                s2_c20(HO - 1)
                for tt in range(n_tt):
                    ysb = yp.tile([P, 512], f32, tag="ysb")
                    # ACT is idle right after the last silu; evicting here
                    # releases the psy0 banks faster than the DVE (which is
                    # still finishing the last GT multiply)
                    nc.scalar.activation(ysb[:], psy0[tt][:, :512], AF.Copy)
                    nc.sync.dma_start(
                        out=out[t0 + tt * P : t0 + (tt + 1) * P, 0:512],
                        in_=ysb[:],
                    )

                psy1 = [
                    ps2.tile([P, TC], f32, tag="psy", name=f"psy1_{tt}")
                    for tt in range(n_tt)
                ]
                for ho in range(HO):
                    if ho < W2RES:
                        w2t = w2r[:, ho, 512:1024]
                    else:
                        w2t = w2p.tile([P, 512], bf16, tag="w2t")
                        nc.sync.dma_start(out=w2t[:], in_=w2[ho, :, 512:1024])
                    for tt in range(n_tt):
                        nc.tensor.matmul(
                            psy1[tt][:, :512],
                            lhsT=gts[:, ho, tt * P : (tt + 1) * P],
                            rhs=w2t[:],
                            start=(ho == 0),
                            stop=(ho == HO - 1),
                        )
                for tt in range(n_tt):
                    ysb = yp.tile([P, 512], f32, tag="ysb")
                    nc.vector.tensor_copy(out=ysb[:], in_=psy1[tt][:, :512])
                    nc.sync.dma_start(
                        out=out[t0 + tt * P : t0 + (tt + 1) * P, 512:1024],
                        in_=ysb[:],
                    )

    nc.compile()
    _cache[key] = nc
    return nc


def kernel(**inputs):
    global last_exec_time_ns
    import jax
    import jax.numpy as jnp
    import ml_dtypes

    from concourse.bass_utils import run_bass_kernel_spmd

    x = inputs["x"]
    Wg = inputs["Wg"]
    bg = inputs["bg"]
    W1 = np.asarray(inputs["W1"], dtype=np.float32)
    b1 = np.asarray(inputs["b1"], dtype=np.float32)
    W3 = np.asarray(inputs["W3"], dtype=np.float32)
    b3 = np.asarray(inputs["b3"], dtype=np.float32)
    W2 = np.asarray(inputs["W2"], dtype=np.float32)
    b2 = np.asarray(inputs["b2"], dtype=np.float32)

    # ---- gate: exact replication of the reference's jnp op sequence ----
    xj = jnp.asarray(x)
    b, t, c = xj.shape
    xf = xj.reshape(b * t, c)
    logits = xf @ jnp.asarray(Wg) + jnp.asarray(bg)
    probs = jax.nn.softmax(logits, axis=-1)
    w, idx = jax.lax.top_k(probs, K)
    w = w / jnp.sum(w, axis=-1, keepdims=True)
    onehot = jax.nn.one_hot(idx, E, dtype=xf.dtype)
    f = onehot.reshape(-1, E).mean(axis=0)
    Pm = probs.mean(axis=0)
    aux = E * jnp.sum(f * Pm)

    idx_np = np.asarray(idx)
    w_np = np.asarray(w, dtype=np.float32)
    aux_np = np.asarray(aux)

    # ---- dispatch (host-side all-to-all) ----
    xf_np = np.asarray(xf, dtype=np.float32)
    xf_bf = xf_np.astype(ml_dtypes.bfloat16)

    ids = []
    wts = []
    for e in range(E):
        sel0 = np.nonzero(idx_np[:, 0] == e)[0]
        sel1 = np.nonzero(idx_np[:, 1] == e)[0]
        ids.append(np.concatenate([sel0, sel1]))
        wts.append(np.concatenate([w_np[sel0, 0], w_np[sel1, 1]]))
    cap = max(len(i) for i in ids)
    cap = max(TC, ((cap + P - 1) // P) * P)

    use_b1 = bool(np.any(b1))
    use_b3 = bool(np.any(b3))
    nc = _build_program(cap, use_b1, use_b3)

    bf = ml_dtypes.bfloat16
    # device-friendly weight layouts (see module docstring)
    W1r = np.ascontiguousarray(
        W1.astype(bf).reshape(E, CO, P, HSL, HS).transpose(0, 3, 2, 1, 4)
    )
    W3r = np.ascontiguousarray(
        W3.astype(bf).reshape(E, CO, P, HSL, HS).transpose(0, 3, 2, 1, 4)
    )
    W2r = W2.astype(bf).reshape(E, HO, P, C)

    in_maps = []
    for e in range(E):
        xt = np.zeros((C, cap), dtype=bf)
        n_e = len(ids[e])
        xt[:, :n_e] = xf_bf[ids[e]].T
        m = {
            "xt": xt.reshape(CO, P, cap),
            "w1": W1r[e],
            "w3": W3r[e],
            "w2": W2r[e],
        }
        if use_b1:
            m["b1"] = b1[e]
        if use_b3:
            m["b3"] = b3[e]
        in_maps.append(m)

    trace = os.environ.get("MOE_TRACE") == "1"
    kw = {}
    if trace:
        kw["trace"] = True
        kw["tmpdir"] = os.environ.get("MOE_TRACE_DIR") or None
    res = run_bass_kernel_spmd(nc, in_maps, list(range(E)), **kw)
    if trace:
        last_exec_time_ns = res.exec_time_ns

    # ---- combine (host-side) ----
    yflat = np.zeros((S, C), dtype=np.float32)
    for e in range(E):
        n_e = len(ids[e])
        contrib = res.results[e]["out"][:n_e]
        if np.any(b2[e]):
            contrib = contrib + b2[e][None, :]
        yflat[ids[e]] += wts[e][:, None] * contrib

    return yflat.reshape(B, T, C), aux_np
